# revision 9
# baseline (speedup 1.0000x reference)
"""Multi-head attention Trainium2 Bass kernel.

Shards across 8 NeuronCores: data-parallel over batch (2) x tensor-parallel
over heads (4 groups of 4 heads). Each core computes its 4 heads' attention
matrix plus a row-parallel partial of the output projection; the host sums
the partials (bfc folded in on the leader core via a zeroed-bias trick on
the others).

Per-core dataflow (S=2048, D=1024, dk=64, 4 heads):
  qT/kT [256,2048] and v [2048,256] projections (fp32r matmuls, biases via
  K=1 rank-1 matmuls) -> per head: scores^T -> exp -> E^T (fp16) -> AV with
  a ones-column appended to V giving softmax denominators for free ->
  recompute scores [i,j] -> exp with bias=-ln(denom) emits normalized
  attention rows directly -> fp32 DMA out. Context is normalized with a
  broadcast divide and fed to the FC partial matmul.
"""

import numpy as np
from contextlib import ExitStack

B = 2
S = 2048
D = 1024
H_PER_CORE = 4
DKC = 256  # dk per core = 4 heads * 64
N_CORES = 8

_CACHED = {}


def _build():
    from concourse import bass, bacc, mybir
    import concourse.tile as tile
    from concourse.alu_op_type import AluOpType

    f32 = mybir.dt.float32
    f32r = mybir.dt.float32r
    f16 = mybir.dt.float16
    AF = mybir.ActivationFunctionType

    nc = bacc.Bacc()

    qt = nc.dram_tensor("qt", [D, S], f16, kind="ExternalInput")
    kt = nc.dram_tensor("kt", [D, S], f16, kind="ExternalInput")
    vt = nc.dram_tensor("vt", [D, S], f16, kind="ExternalInput")
    wq = nc.dram_tensor("wq", [D, DKC], f16, kind="ExternalInput")
    wk = nc.dram_tensor("wk", [D, DKC], f16, kind="ExternalInput")
    wv = nc.dram_tensor("wv", [D, DKC], f16, kind="ExternalInput")
    wfc = nc.dram_tensor("wfc", [DKC, D], f16, kind="ExternalInput")
    bq = nc.dram_tensor("bq", [1, DKC], f16, kind="ExternalInput")
    bk = nc.dram_tensor("bk", [1, DKC], f16, kind="ExternalInput")
    bv = nc.dram_tensor("bv", [1, DKC], f16, kind="ExternalInput")
    bfc = nc.dram_tensor("bfc", [1, D], f16, kind="ExternalInput")
    a_out = nc.dram_tensor("a_out", [H_PER_CORE, S, S], f32, kind="ExternalOutput")
    p_out = nc.dram_tensor("p_out", [S, D], f32, kind="ExternalOutput")
    scr_den = nc.dram_tensor("scr_den", [H_PER_CORE, S], f32)  # denominator bounce
    scr_inv = nc.dram_tensor("scr_inv", [H_PER_CORE, S], f32)  # 1/denominator

    NSEQT = S // 128   # 16 seq tiles of 128
    NBLK = S // 512    # 4 blocks of 512
    DC = D // 128      # 8 contraction chunks

    def r(ap):
        return ap

    with ExitStack() as ctx:
        tc = ctx.enter_context(tile.TileContext(nc))
        const = ctx.enter_context(tc.tile_pool(name="const", bufs=1))
        persist = ctx.enter_context(tc.tile_pool(name="persist", bufs=1))
        psum = ctx.enter_context(tc.tile_pool(name="psum", bufs=2, space="PSUM"))

        ones = const.tile([1, 512], f16)
        nc.vector.memset(ones, 1.0)

        # persistent SBUF residents
        qT_sb = [persist.tile([128, S], f16, tag=f"qT{t}", name=f"qT{t}") for t in range(2)]
        kT_sb = [persist.tile([128, S], f16, tag=f"kT{t}", name=f"kT{t}") for t in range(2)]
        v_sb = [persist.tile([128, 65 * H_PER_CORE], f16, tag=f"v{st}", name=f"v{st}")
                for st in range(NSEQT)]
        ctx_h = [persist.tile([64, S], f16, tag=f"ctx{h}", name=f"ctx{h}") for h in range(H_PER_CORE)]
        wfc_h = [persist.tile([64, D], f16, tag=f"wfc{h}", name=f"wfc{h}") for h in range(H_PER_CORE)]
        bfc_sb = persist.tile([1, D], f16, tag="bfc")
        negln = persist.tile([128, 16 * H_PER_CORE], f32, tag="negln")

        for h in range(H_PER_CORE):
            nc.sync.dma_start(out=wfc_h[h], in_=wfc[h * 64:(h + 1) * 64, :])
        nc.sync.dma_start(out=bfc_sb, in_=bfc[:, :])

        # ---------------- projections ----------------
        with tc.tile_pool(name="proj", bufs=1) as proj:
            w_sb = {}
            b_sb = {}
            for name, wsrc, bsrc in (("q", wq, bq), ("k", wk, bk), ("v", wv, bv)):
                w_sb[name] = proj.tile([128, DC, DKC], f16, tag=f"w{name}", name=f"w{name}")
                nc.sync.dma_start(
                    out=w_sb[name],
                    in_=wsrc[:, :].rearrange("(c p) n -> p c n", p=128),
                )
                b_sb[name] = proj.tile([1, DKC], f16, tag=f"b{name}", name=f"b{name}")
                nc.sync.dma_start(out=b_sb[name], in_=bsrc[:, :])

            # q/k: out tiles [dk 128, seq] = sum_c W[c,:,dk].T @ inT[c,:,:]
            for name, insrc, outsb in (("q", qt, qT_sb), ("k", kt, kT_sb)):
                chunks = []
                for c in range(DC):
                    t = proj.tile([128, S], f16, tag=f"in{c}", name=f"in{c}")
                    nc.sync.dma_start(out=t, in_=insrc[c * 128:(c + 1) * 128, :])
                    chunks.append(t)
                for dk in range(2):
                    ps = psum.tile([128, S], f32, tag="mm")
                    for sb_i in range(NBLK):
                        sl = slice(sb_i * 512, (sb_i + 1) * 512)
                        for c in range(DC):
                            nc.tensor.matmul(
                                ps[:, sl],
                                r(w_sb[name][:, c, dk * 128:(dk + 1) * 128]),
                                r(chunks[c][:, sl]),
                                start=(c == 0), stop=False,
                            )
                        nc.tensor.matmul(
                            ps[:, sl],
                            r(b_sb[name][:, dk * 128:(dk + 1) * 128]),
                            r(ones[:, 0:512]),
                            start=False, stop=True,
                        )
                    nc.vector.tensor_copy(outsb[dk], ps)

            # v: out tiles [seq 128, dk 256] = sum_c vtT[c, seq].T @ Wv[c]
            vchunks = []
            for c in range(DC):
                t = proj.tile([128, S], f16, tag=f"in{c}", name=f"in{c}")
                nc.sync.dma_start(out=t, in_=vt[c * 128:(c + 1) * 128, :])
                vchunks.append(t)
            for st in range(NSEQT):
                ssl = slice(st * 128, (st + 1) * 128)
                ps = psum.tile([128, DKC], f32, tag="mm")
                for c in range(DC):
                    nc.tensor.matmul(
                        ps, r(vchunks[c][:, ssl]), r(w_sb["v"][:, c, :]),
                        start=(c == 0), stop=False,
                    )
                nc.tensor.matmul(
                    ps, r(ones[:, 0:128]), r(b_sb["v"]), start=False, stop=True,
                )
                # pack 4 heads with a ones column after each head's 64 cols
                nc.vector.memset(v_sb[st], 1.0)
                for h in range(H_PER_CORE):
                    nc.vector.tensor_copy(
                        v_sb[st][:, h * 65:h * 65 + 64],
                        ps[:, h * 64:(h + 1) * 64],
                    )

        # ---------------- attention ----------------
        with tc.tile_pool(name="attn", bufs=1) as attn, \
             tc.tile_pool(name="astream", bufs=1) as astream:
            for hp in range(H_PER_CORE // 2):
                pair = (2 * hp, 2 * hp + 1)
                for h in pair:
                    t_idx, row0 = h // 2, (h % 2) * 64
                    ksl = kT_sb[t_idx]
                    qsl = qT_sb[t_idx]
                    # scores^T [j, i] -> exp -> E^T fp16
                    et = [attn.tile([128, S], f16, tag=f"et{jc}", name=f"et{jc}")
                          for jc in range(NSEQT)]
                    for jc in range(NSEQT):
                        ps = psum.tile([128, S], f32, tag="mm")
                        for ib in range(NBLK):
                            isl = slice(ib * 512, (ib + 1) * 512)
                            nc.tensor.matmul(
                                ps[:, isl],
                                r(ksl[row0:row0 + 64, jc * 128:(jc + 1) * 128]),
                                r(qsl[row0:row0 + 64, isl]),
                                start=True, stop=True,
                            )
                        nc.scalar.activation(et[jc], ps, AF.Exp, scale=0.125)
                    # AV: ctxT_aug [65, i] = sum_j V_aug[j, 65].T @ E^T[j, i]
                    for ib in range(NBLK):
                        isl = slice(ib * 512, (ib + 1) * 512)
                        av = psum.tile([65, 512], f32, tag="mm")
                        for jc in range(NSEQT):
                            nc.tensor.matmul(
                                av,
                                v_sb[jc][:, h * 65:(h + 1) * 65],
                                et[jc][:, isl],
                                start=(jc == 0), stop=(jc == 15),
                            )
                        nc.vector.tensor_copy(ctx_h[h][:, isl], av[0:64, :])
                        dr = attn.tile([65, 512], f32, tag="denrow", bufs=2,
                                       name="denrow")
                        nc.vector.tensor_copy(dr[64:65, :], av[64:65, :])
                        nc.sync.dma_start(
                            out=scr_den[h, ib * 512:(ib + 1) * 512],
                            in_=dr[64:65, :],
                        )
                # denominators -> -ln(denom) in [i-part, (h, it)] layout
                dn = attn.tile([128, 16 * 2], f32, tag="dn")
                nc.sync.dma_start(
                    out=dn,
                    in_=scr_den[pair[0]:pair[1] + 1, :].rearrange(
                        "h (it p) -> p (h it)", p=128),
                )
                lnt = attn.tile([128, 16 * 2], f32, tag="lnt")
                nc.scalar.activation(lnt, dn, AF.Ln)
                nc.vector.tensor_scalar_mul(
                    negln[:, pair[0] * 16:(pair[1] + 1) * 16], lnt, -1.0)
                inv = attn.tile([128, 16 * 2], f32, tag="inv")
                nc.vector.reciprocal(inv, dn)
                nc.gpsimd.dma_start(
                    out=scr_inv[pair[0]:pair[1] + 1, :].rearrange(
                        "h (it p) -> p (h it)", p=128),
                    in_=inv)
                # recompute scores [i, j] -> exp(0.125*s - ln(denom)) -> A rows
                for it in range(NSEQT):
                    for h in pair:
                        t_idx, row0 = h // 2, (h % 2) * 64
                        ps = psum.tile([128, S], f32, tag="mm")
                        for jb in range(NBLK):
                            jsl = slice(jb * 512, (jb + 1) * 512)
                            nc.tensor.matmul(
                                ps[:, jsl],
                                r(qT_sb[t_idx][row0:row0 + 64,
                                               it * 128:(it + 1) * 128]),
                                r(kT_sb[t_idx][row0:row0 + 64, jsl]),
                                start=True, stop=True,
                            )
                        a_t = astream.tile([128, S], f32, tag="a", bufs=3)
                        nc.scalar.activation(
                            a_t, ps, AF.Exp,
                            bias=negln[:, h * 16 + it:h * 16 + it + 1],
                            scale=0.125,
                        )
                        nc.sync.dma_start(
                            out=a_out[h, it * 128:(it + 1) * 128, :], in_=a_t)

        # ---------------- context normalize + FC ----------------
        with tc.tile_pool(name="fc", bufs=1) as fc:
            for h in range(H_PER_CORE):
                bc = fc.tile([64, S], f32, tag=f"bc{h % 2}", bufs=2)
                nc.gpsimd.dma_start(
                    out=bc, in_=scr_inv[h:h + 1, :].to_broadcast([64, S]))
                nc.vector.tensor_tensor(
                    ctx_h[h], ctx_h[h], bc, op=AluOpType.mult)
            for st in range(NSEQT):
                ssl = slice(st * 128, (st + 1) * 128)
                ps = psum.tile([128, D], f32, tag="mm")
                for nb in range(2):
                    nsl = slice(nb * 512, (nb + 1) * 512)
                    for h in range(H_PER_CORE):
                        nc.tensor.matmul(
                            ps[:, nsl],
                            r(ctx_h[h][:, ssl]),
                            r(wfc_h[h][:, nsl]),
                            start=(h == 0), stop=False,
                        )
                    nc.tensor.matmul(
                        ps[:, nsl], r(ones[:, 0:128]), r(bfc_sb[:, nsl]),
                        start=False, stop=True,
                    )
                ot = fc.tile([128, D], f32, tag="fcout", bufs=3)
                nc.vector.tensor_copy(ot, ps)
                nc.sync.dma_start(out=p_out[ssl, :], in_=ot)

    nc.compile()
    return nc


def _get_nc():
    if "nc" not in _CACHED:
        _CACHED["nc"] = _build()
    return _CACHED["nc"]


def _make_in_maps(Q, K, V, Wq, bq, Wk, bk, Wv, bv, Wfc, bfc):
    in_maps = []
    for core in range(N_CORES):
        b, g = divmod(core, 4)
        hs = slice(g * DKC, (g + 1) * DKC)
        in_maps.append({
            "qt": np.ascontiguousarray(np.asarray(Q[b], np.float16).T),
            "kt": np.ascontiguousarray(np.asarray(K[b], np.float16).T),
            "vt": np.ascontiguousarray(np.asarray(V[b], np.float16).T),
            "wq": np.ascontiguousarray(np.asarray(Wq, np.float16)[:, hs]),
            "wk": np.ascontiguousarray(np.asarray(Wk, np.float16)[:, hs]),
            "wv": np.ascontiguousarray(np.asarray(Wv, np.float16)[:, hs]),
            "wfc": np.ascontiguousarray(np.asarray(Wfc, np.float16)[hs, :]),
            "bq": np.asarray(bq, np.float16)[None, hs],
            "bk": np.asarray(bk, np.float16)[None, hs],
            "bv": np.asarray(bv, np.float16)[None, hs],
            "bfc": (np.asarray(bfc, np.float16) if g == 0
                    else np.zeros(D, np.float16))[None, :],
        })
    return in_maps


def _assemble(res):
    attention = np.empty((B, 16, S, S), np.float32)
    output = np.zeros((B, S, D), np.float32)
    for core in range(N_CORES):
        b, g = divmod(core, 4)
        attention[b, g * H_PER_CORE:(g + 1) * H_PER_CORE] = res[core]["a_out"]
        output[b] += res[core]["p_out"]
    return output, attention


def kernel(Q, K, V, Wq, bq, Wk, bk, Wv, bv, Wfc, bfc):
    from concourse.bass_utils import run_bass_kernel_spmd

    nc = _get_nc()
    in_maps = _make_in_maps(Q, K, V, Wq, bq, Wk, bk, Wv, bv, Wfc, bfc)
    res = run_bass_kernel_spmd(nc, in_maps, list(range(N_CORES))).results
    return _assemble(res)


# revision 11
# speedup vs baseline: 1.0400x; 1.0400x over previous
"""Multi-head attention Trainium2 Bass kernel.

Shards across 8 NeuronCores: data-parallel over batch (2) x tensor-parallel
over heads (4 groups of 4 heads). Each core computes its 4 heads' attention
matrix plus a row-parallel partial of the output projection; the host sums
the partials (bfc folded in on the leader core via a zeroed-bias trick on
the others).

Per-core dataflow (S=2048, D=1024, dk=64, 4 heads, fp16 operands / fp32
accumulation):
  qT/kT [256,2048] and v [2048,256] projections (biases via K=1 rank-1
  matmuls) -> per head, fused over j-chunks: scores^T tile -> exp ->
  E^T (fp16, small ring) -> immediately accumulated into AV psums, with a
  ones-column appended to V so softmax denominators fall out of the same
  matmul -> recompute scores [i,j] -> exp with bias=-ln(denom) emits
  normalized attention rows directly -> fp32 DMA out. Context rows are
  normalized with a broadcast multiply by 1/denom and fed to the FC
  partial matmul.
"""

import numpy as np
from contextlib import ExitStack

B = 2
S = 2048
D = 1024
H_PER_CORE = 4
DKC = 256  # dk per core = 4 heads * 64
N_CORES = 8

_CACHED = {}


def _build():
    from concourse import bass, bacc, mybir
    import concourse.tile as tile
    from concourse.alu_op_type import AluOpType

    f32 = mybir.dt.float32
    f16 = mybir.dt.float16
    AF = mybir.ActivationFunctionType

    nc = bacc.Bacc()

    qt = nc.dram_tensor("qt", [D, S], f16, kind="ExternalInput")
    kt = nc.dram_tensor("kt", [D, S], f16, kind="ExternalInput")
    vt = nc.dram_tensor("vt", [D, S], f16, kind="ExternalInput")
    wq = nc.dram_tensor("wq", [D, DKC], f16, kind="ExternalInput")
    wk = nc.dram_tensor("wk", [D, DKC], f16, kind="ExternalInput")
    wv = nc.dram_tensor("wv", [D, DKC], f16, kind="ExternalInput")
    wfc = nc.dram_tensor("wfc", [DKC, D], f16, kind="ExternalInput")
    bq = nc.dram_tensor("bq", [1, DKC], f16, kind="ExternalInput")
    bk = nc.dram_tensor("bk", [1, DKC], f16, kind="ExternalInput")
    bv = nc.dram_tensor("bv", [1, DKC], f16, kind="ExternalInput")
    bfc = nc.dram_tensor("bfc", [1, D], f16, kind="ExternalInput")
    a_out = nc.dram_tensor("a_out", [H_PER_CORE, S, S], f32, kind="ExternalOutput")
    p_out = nc.dram_tensor("p_out", [S, D], f32, kind="ExternalOutput")
    scr_den = nc.dram_tensor("scr_den", [H_PER_CORE, S], f32)  # denominators
    scr_inv = nc.dram_tensor("scr_inv", [H_PER_CORE, S], f32)  # 1/denominator

    NSEQT = S // 128   # 16 seq tiles of 128
    NBLK = S // 512    # 4 blocks of 512
    DC = D // 128      # 8 contraction chunks

    with ExitStack() as ctx:
        tc = ctx.enter_context(tile.TileContext(nc))
        const = ctx.enter_context(tc.tile_pool(name="const", bufs=1))
        persist = ctx.enter_context(tc.tile_pool(name="persist", bufs=1))
        psum = ctx.enter_context(tc.tile_pool(name="psum", bufs=2, space="PSUM"))

        ones = const.tile([1, 512], f16)
        nc.vector.memset(ones, 1.0)

        # persistent SBUF residents
        qT_sb = [persist.tile([128, S], f16, tag=f"qT{t}", name=f"qT{t}")
                 for t in range(2)]
        kT_sb = [persist.tile([128, S], f16, tag=f"kT{t}", name=f"kT{t}")
                 for t in range(2)]
        v_sb = [persist.tile([128, 65 * H_PER_CORE], f16, tag=f"v{st}",
                             name=f"v{st}") for st in range(NSEQT)]
        ctx_sb = [persist.tile([128, S], f16, tag=f"ctx{t}", name=f"ctx{t}")
                  for t in range(2)]
        wfc_sb = [persist.tile([128, D], f16, tag=f"wfc{t}", name=f"wfc{t}")
                  for t in range(2)]
        bfc_sb = persist.tile([1, D], f16, tag="bfc")
        negln = persist.tile([128, 16 * H_PER_CORE], f32, tag="negln")

        for t in range(2):
            nc.sync.dma_start(out=wfc_sb[t], in_=wfc[t * 128:(t + 1) * 128, :])
        nc.sync.dma_start(out=bfc_sb, in_=bfc[:, :])

        # ---------------- projections ----------------
        with tc.tile_pool(name="proj", bufs=1) as proj:
            w_sb = {}
            b_sb = {}
            for name, wsrc, bsrc in (("q", wq, bq), ("k", wk, bk), ("v", wv, bv)):
                w_sb[name] = proj.tile([128, DC, DKC], f16, tag=f"w{name}",
                                       name=f"w{name}")
                nc.sync.dma_start(
                    out=w_sb[name],
                    in_=wsrc[:, :].rearrange("(c p) n -> p c n", p=128),
                )
                b_sb[name] = proj.tile([1, DKC], f16, tag=f"b{name}",
                                       name=f"b{name}")
                nc.sync.dma_start(out=b_sb[name], in_=bsrc[:, :])

            # q/k: out tiles [dk 128, seq] = sum_c W[c,:,dk].T @ inT[c,:,:]
            for name, insrc, outsb in (("q", qt, qT_sb), ("k", kt, kT_sb)):
                chunks = []
                for c in range(DC):
                    t = proj.tile([128, S], f16, tag=f"in{c}", name=f"in{c}")
                    nc.sync.dma_start(out=t, in_=insrc[c * 128:(c + 1) * 128, :])
                    chunks.append(t)
                for dk in range(2):
                    for sbp in range(2):
                        ps = psum.tile([128, 1024], f32, tag="st", name="ps")
                        for sb_i in range(2):
                            po = slice(sb_i * 512, (sb_i + 1) * 512)
                            sl = slice(sbp * 1024 + sb_i * 512,
                                       sbp * 1024 + (sb_i + 1) * 512)
                            for c in range(DC):
                                nc.tensor.matmul(
                                    ps[:, po],
                                    w_sb[name][:, c, dk * 128:(dk + 1) * 128],
                                    chunks[c][:, sl],
                                    start=(c == 0), stop=False,
                                )
                            nc.tensor.matmul(
                                ps[:, po],
                                b_sb[name][:, dk * 128:(dk + 1) * 128],
                                ones[:, 0:512],
                                start=False, stop=True,
                            )
                        nc.vector.tensor_copy(
                            outsb[dk][:, sbp * 1024:(sbp + 1) * 1024], ps)

            # v: out tiles [seq 128, dk 256] = sum_c vtT[c, seq].T @ Wv[c]
            vchunks = []
            for c in range(DC):
                t = proj.tile([128, S], f16, tag=f"in{c}", name=f"in{c}")
                nc.sync.dma_start(out=t, in_=vt[c * 128:(c + 1) * 128, :])
                vchunks.append(t)
            for st in range(NSEQT):
                ssl = slice(st * 128, (st + 1) * 128)
                ps = psum.tile([128, DKC], f32, tag="st", name="ps")
                for c in range(DC):
                    nc.tensor.matmul(
                        ps, vchunks[c][:, ssl], w_sb["v"][:, c, :],
                        start=(c == 0), stop=False,
                    )
                nc.tensor.matmul(
                    ps, ones[:, 0:128], b_sb["v"], start=False, stop=True,
                )
                # pack 4 heads with a ones column after each head's 64 cols
                nc.vector.memset(v_sb[st], 1.0)
                for h in range(H_PER_CORE):
                    nc.vector.tensor_copy(
                        v_sb[st][:, h * 65:h * 65 + 64],
                        ps[:, h * 64:(h + 1) * 64],
                    )

        # ---------------- attention ----------------
        with tc.tile_pool(name="attn", bufs=1) as attn, \
             tc.tile_pool(name="astream", bufs=1) as astream:
            for hp in range(H_PER_CORE // 2):
                pair = (2 * hp, 2 * hp + 1)
                for h in pair:
                    t_idx, row0 = h // 2, (h % 2) * 64
                    ksl = kT_sb[t_idx]
                    qsl = qT_sb[t_idx]
                    # fused: scores^T tile -> exp -> E^T ring -> AV accum
                    avs = [psum.tile([65, 512], f32, tag=f"av{ib}",
                                     bufs=1, name=f"av{ib}")
                           for ib in range(NBLK)]
                    pend = None  # (et tile, jc, ih) awaiting AV matmuls
                    for jc in range(NSEQT):
                        for ih in range(2):
                            ps = psum.tile([128, 1024], f32, tag="st",
                                           name="ps")
                            for q2 in range(2):
                                isl = slice(ih * 1024 + q2 * 512,
                                            ih * 1024 + (q2 + 1) * 512)
                                nc.tensor.matmul(
                                    ps[:, q2 * 512:(q2 + 1) * 512],
                                    ksl[row0:row0 + 64, jc * 128:(jc + 1) * 128],
                                    qsl[row0:row0 + 64, isl],
                                    start=True, stop=True,
                                )
                            et_t = attn.tile([128, 1024], f16, tag="et",
                                             bufs=4, name="et")
                            nc.scalar.activation(et_t, ps, AF.Exp, scale=0.125)
                            if pend is not None:
                                pet, pjc, pih = pend
                                for q2 in range(2):
                                    nc.tensor.matmul(
                                        avs[pih * 2 + q2],
                                        v_sb[pjc][:, h * 65:(h + 1) * 65],
                                        pet[:, q2 * 512:(q2 + 1) * 512],
                                        start=(pjc == 0), stop=(pjc == 15),
                                    )
                            pend = (et_t, jc, ih)
                    pet, pjc, pih = pend
                    for q2 in range(2):
                        nc.tensor.matmul(
                            avs[pih * 2 + q2],
                            v_sb[pjc][:, h * 65:(h + 1) * 65],
                            pet[:, q2 * 512:(q2 + 1) * 512],
                            start=(pjc == 0), stop=(pjc == 15),
                        )
                    for ib in range(NBLK):
                        isl = slice(ib * 512, (ib + 1) * 512)
                        nc.vector.tensor_copy(
                            ctx_sb[t_idx][row0:row0 + 64, isl],
                            avs[ib][0:64, :])
                        dr = attn.tile([65, 512], f32, tag="denrow", bufs=2,
                                       name="denrow")
                        nc.vector.tensor_copy(dr[64:65, :], avs[ib][64:65, :])
                        nc.sync.dma_start(
                            out=scr_den[h, ib * 512:(ib + 1) * 512],
                            in_=dr[64:65, :],
                        )
                # denominators -> -ln(denom), 1/denom in [i-part, (h, it)]
                dn = attn.tile([128, 16 * 2], f32, tag="dn")
                nc.sync.dma_start(
                    out=dn,
                    in_=scr_den[pair[0]:pair[1] + 1, :].rearrange(
                        "h (it p) -> p (h it)", p=128),
                )
                lnt = attn.tile([128, 16 * 2], f32, tag="lnt")
                nc.scalar.activation(lnt, dn, AF.Ln)
                nc.vector.tensor_scalar_mul(
                    negln[:, pair[0] * 16:(pair[1] + 1) * 16], lnt, -1.0)
                inv = attn.tile([128, 16 * 2], f32, tag="inv")
                nc.vector.reciprocal(inv, dn)
                nc.gpsimd.dma_start(
                    out=scr_inv[pair[0]:pair[1] + 1, :].rearrange(
                        "h (it p) -> p (h it)", p=128),
                    in_=inv)
                # recompute scores [i, j] -> exp(0.125*s - ln(denom)) -> A
                for it in range(NSEQT):
                    for h in pair:
                        t_idx, row0 = h // 2, (h % 2) * 64
                        for jh in range(2):
                            ps = psum.tile([128, 1024], f32, tag="st",
                                           name="ps")
                            for q2 in range(2):
                                jsl = slice(jh * 1024 + q2 * 512,
                                            jh * 1024 + (q2 + 1) * 512)
                                nc.tensor.matmul(
                                    ps[:, q2 * 512:(q2 + 1) * 512],
                                    qT_sb[t_idx][row0:row0 + 64,
                                                 it * 128:(it + 1) * 128],
                                    kT_sb[t_idx][row0:row0 + 64, jsl],
                                    start=True, stop=True,
                                )
                            a_t = astream.tile([128, 1024], f32, tag="a",
                                               bufs=4, name="a_t")
                            nc.scalar.activation(
                                a_t, ps, AF.Exp,
                                bias=negln[:, h * 16 + it:h * 16 + it + 1],
                                scale=0.125,
                            )
                            nc.sync.dma_start(
                                out=a_out[h, it * 128:(it + 1) * 128,
                                          jh * 1024:(jh + 1) * 1024],
                                in_=a_t)

        # ---------------- context normalize + FC ----------------
        with tc.tile_pool(name="fc", bufs=1) as fc:
            for t in range(2):
                bc = fc.tile([128, S], f32, tag="bc", bufs=2, name="bc")
                for half in range(2):
                    h = 2 * t + half
                    nc.gpsimd.dma_start(
                        out=bc[half * 64:(half + 1) * 64, :],
                        in_=scr_inv[h:h + 1, :].to_broadcast([64, S]))
                nc.vector.tensor_tensor(
                    ctx_sb[t], ctx_sb[t], bc, op=AluOpType.mult)
            for st in range(NSEQT):
                ssl = slice(st * 128, (st + 1) * 128)
                ps = psum.tile([128, D], f32, tag="st", name="ps")
                for nb in range(2):
                    nsl = slice(nb * 512, (nb + 1) * 512)
                    for t in range(2):
                        nc.tensor.matmul(
                            ps[:, nsl],
                            ctx_sb[t][:, ssl],
                            wfc_sb[t][:, nsl],
                            start=(t == 0), stop=False,
                        )
                    nc.tensor.matmul(
                        ps[:, nsl], ones[:, 0:128], bfc_sb[:, nsl],
                        start=False, stop=True,
                    )
                ot = fc.tile([128, D], f32, tag="fcout", bufs=3, name="ot")
                nc.vector.tensor_copy(ot, ps)
                nc.sync.dma_start(out=p_out[ssl, :], in_=ot)

    nc.compile()
    return nc


def _get_nc():
    if "nc" not in _CACHED:
        _CACHED["nc"] = _build()
    return _CACHED["nc"]


def _make_in_maps(Q, K, V, Wq, bq, Wk, bk, Wv, bv, Wfc, bfc):
    in_maps = []
    for core in range(N_CORES):
        b, g = divmod(core, 4)
        hs = slice(g * DKC, (g + 1) * DKC)
        in_maps.append({
            "qt": np.ascontiguousarray(np.asarray(Q[b], np.float16).T),
            "kt": np.ascontiguousarray(np.asarray(K[b], np.float16).T),
            "vt": np.ascontiguousarray(np.asarray(V[b], np.float16).T),
            "wq": np.ascontiguousarray(np.asarray(Wq, np.float16)[:, hs]),
            "wk": np.ascontiguousarray(np.asarray(Wk, np.float16)[:, hs]),
            "wv": np.ascontiguousarray(np.asarray(Wv, np.float16)[:, hs]),
            "wfc": np.ascontiguousarray(np.asarray(Wfc, np.float16)[hs, :]),
            "bq": np.asarray(bq, np.float16)[None, hs],
            "bk": np.asarray(bk, np.float16)[None, hs],
            "bv": np.asarray(bv, np.float16)[None, hs],
            "bfc": (np.asarray(bfc, np.float16) if g == 0
                    else np.zeros(D, np.float16))[None, :],
        })
    return in_maps


def _assemble(res):
    attention = np.empty((B, 16, S, S), np.float32)
    output = np.zeros((B, S, D), np.float32)
    for core in range(N_CORES):
        b, g = divmod(core, 4)
        attention[b, g * H_PER_CORE:(g + 1) * H_PER_CORE] = res[core]["a_out"]
        output[b] += res[core]["p_out"]
    return output, attention


def kernel(Q, K, V, Wq, bq, Wk, bk, Wv, bv, Wfc, bfc):
    from concourse.bass_utils import run_bass_kernel_spmd

    nc = _get_nc()
    in_maps = _make_in_maps(Q, K, V, Wq, bq, Wk, bk, Wv, bv, Wfc, bfc)
    res = run_bass_kernel_spmd(nc, in_maps, list(range(N_CORES))).results
    return _assemble(res)
